# revision 10
# baseline (speedup 1.0000x reference)
"""AriaGroupedGEMM (MoE grouped GEMM) on 8 TRN2 NeuronCores.

Problem: input [4096, 2048] f32, weight [8, 2048, 2048] f32,
tokens_per_expert [8] int32 (tokens pre-sorted by expert).
out[i] = input[i] @ weight[expert_of(i)].

Strategy: expert-parallel. Core g owns expert g's weight and its token
group (boundaries computed on host from tokens_per_expert). Each core
runs a dense [T_pad, 2048] @ [2048, 2048] GEMM in bf16 (fp32 PSUM
accumulation): 256 matmuls of [128x128]@[128x512] = 54.6us of PE
streaming at the warm 2.4GHz back-to-back rate -- the compute floor.

Raw bacc (no TileContext), manual semaphores. Each HWDGE dma_start
occupies its sequencer ~0.65us and its completion semaphore lags the
last byte by ~2.5us (HBM receipt + engine start), so the input stream
is one FIFO on the sync ring in exact consumption order: phase A
interleaves n-blocks 0/1 at k-chunk granularity across all 8 PSUM
banks (each arriving 512KB chunk unlocks 16 matmuls = 3.4us of PE
work), phase B (blocks 2/3) prefetches far ahead as two 2MB DMAs and
runs dense m-major bursts. Enough thin warm-up matmuls run over
scratch to put >3.4us of PE busy-time before the first real matmul,
so the HAM clock gate is fully lifted (2.4GHz) when data lands.
Outputs stage through 8 SBUF tiles onto the scalar ring as contiguous
128KB blocks; the fixed walrus NEFF epilogue (~6.5us of per-semaphore
resets) runs long past the last output's completion receipt, so no
end-of-kernel completion waits are needed.
"""
import sys
import functools

for _p in ("/opt/trn_rl_repo", "/root/.axon_site/_ro/trn_rl_repo"):
    if _p not in sys.path:
        sys.path.insert(0, _p)

import numpy as np
import ml_dtypes

import concourse.mybir as mybir
from concourse import bacc
from concourse import bass_utils

P = 128
K = 2048            # in_features (contraction)
N = 2048            # out_features
G = 8               # experts == cores
KO = K // P         # 16 k-subtiles
BW = 512            # n-block width (one PSUM bank of fp32)
NBLK = N // BW      # 4 n-blocks

COMPUTE_DT = mybir.dt.bfloat16
NP_COMPUTE = ml_dtypes.bfloat16
OUT_DT = mybir.dt.bfloat16      # psum(f32) -> bf16 on the way out; host upcasts

N_WARMUP_MM = 40    # thin N=128 warm-up matmuls; >3.4us of PE busy lifts HAM
N_OSB = 8           # output staging tiles in SBUF


@functools.lru_cache(maxsize=4)
def _build(t_pad: int):
    """Build + compile the per-core GEMM graph for token-pad t_pad."""
    mt = t_pad // P  # m tiles of 128 tokens

    nc = bacc.Bacc("TRN2", target_bir_lowering=False, debug=False)

    # host-swizzled DRAM layouts (fully contiguous per DMA):
    # xt[mi, p, ko*P + j] = X[mi*P + j, ko*P + p]
    # w[b, p, ko*BW + j]  = W[ko*P + p, b*BW + j]
    # out[b, t, j]        = OUT[t, b*BW + j]
    xt_d = nc.dram_tensor(
        "xt", [mt, P, KO * P], COMPUTE_DT, kind="ExternalInput").ap()
    w_d = nc.dram_tensor(
        "w", [NBLK, P, KO * BW], COMPUTE_DT, kind="ExternalInput").ap()
    out_d = nc.dram_tensor(
        "out", [NBLK, t_pad, BW], OUT_DT, kind="ExternalOutput").ap()

    # SBUF
    xt_sb = [nc.alloc_sbuf_tensor(f"xt_sb{m}", [P, KO * P], COMPUTE_DT).ap()
             for m in range(mt)]
    w_sb = [nc.alloc_sbuf_tensor(f"w_sb{b}", [P, KO * BW], COMPUTE_DT).ap()
            for b in range(NBLK)]
    o_sb = [nc.alloc_sbuf_tensor(f"o_sb{i}", [P, BW], OUT_DT).ap()
            for i in range(N_OSB)]
    wu_lhs = nc.alloc_sbuf_tensor("wu_lhs", [P, P], COMPUTE_DT).ap()
    wu_rhs = nc.alloc_sbuf_tensor("wu_rhs", [P, P], COMPUTE_DT).ap()

    # PSUM: 8 banks; phase A owns all of them as (b, m) -> 4b+m for
    # b in {0,1}; phase B reuses bank (b-2)*4+m after its copy drains.
    # Warm-ups hit bank 7, whose first real tenant starts much later.
    pk = [nc.alloc_psum_tensor(f"pk{j}", [P, BW], mybir.dt.float32).ap()
          for j in range(8)]
    wu_ps = pk[7][:, :P]

    NG = NBLK * mt  # real matmul groups

    pe_sem = nc.alloc_semaphore("pe_sem")   # PE group-final matmul done
    cp_sem = nc.alloc_semaphore("cp_sem")   # DVE psum->sbuf copy done
    od = [nc.alloc_semaphore(f"od{g}") for g in range(NG)]  # out DMA done

    # ---- sync ring: input DMAs in exact consumption order, one sem each
    dsem = []

    def dma_in(dst_ap, src_ap, tag):
        s = nc.alloc_semaphore(f"d{len(dsem)}_{tag}")
        nc.sync.dma_start(dst_ap, src_ap).then_inc(s, 16)
        dsem.append(s)
        return len(dsem) - 1

    def load_xt(m, k0=0, k1=KO):
        return dma_in(xt_sb[m][:, k0 * P:k1 * P],
                      xt_d[m][:, k0 * P:k1 * P], f"xt{m}_{k0}")

    def load_w(b, k0, k1):
        return dma_in(w_sb[b][:, k0 * BW:k1 * BW],
                      w_d[b][:, k0 * BW:k1 * BW], f"w{b}_{k0}")

    # ---- PE stream helpers
    waited = set()

    def pe_wait(sem_id):
        if sem_id is not None and sem_id not in waited:
            nc.tensor.wait_ge(dsem[sem_id], 16)
            waited.add(sem_id)

    def mm(bank, m, b, ko):
        ins = nc.tensor.matmul(
            pk[bank],
            xt_sb[m][:, ko * P:(ko + 1) * P],
            w_sb[b][:, ko * BW:(ko + 1) * BW],
            start=(ko == 0),
            stop=(ko == KO - 1),
        )
        if ko == KO - 1:
            ins.then_inc(pe_sem, 1)

    # warm-ups: matmuls over uninitialized scratch (result never read);
    # no deps at all, so the PE is busy from the first instant
    for _ in range(N_WARMUP_MM):
        nc.tensor.matmul(wu_ps, wu_lhs, wu_rhs, start=True, stop=True,
                         skip_group_check=True)

    if mt == 4:
        # DMA order = PE consumption order (phase A interleaves b0/b1)
        d_xt0a = load_xt(0, 0, 8)
        d_w0a = load_w(0, 0, 2)
        d_w0b = load_w(0, 2, 4)
        d_xt0b = load_xt(0, 8, 16)
        d_xt = {1: load_xt(1), 0: None}
        d_w1a = load_w(1, 0, 4)
        d_xt[2] = load_xt(2)
        d_xt[3] = load_xt(3)
        d_wA = {}  # (b, ci) -> sem, chunks k4-9, k10-15 for b0/b1
        for ci, (k0, k1) in enumerate([(4, 10), (10, 16)]):
            d_wA[(0, ci)] = load_w(0, k0, k1)
            d_wA[(1, ci)] = load_w(1, k0, k1)
        d_b2 = load_w(2, 0, 16)
        d_b3 = load_w(3, 0, 16)

        # phase A waves: (waits, [(m, b, ko-range)])
        waves = [
            ([d_xt0a, d_w0a], [(0, 0, 0, 2)]),
            ([d_w0b],         [(0, 0, 2, 4)]),
            ([d_xt[1]],       [(1, 0, 0, 4)]),
            ([d_w1a],         [(0, 1, 0, 4), (1, 1, 0, 4)]),
            ([d_xt0b],        []),
            ([d_xt[2]],       [(2, 0, 0, 4), (2, 1, 0, 4)]),
            ([d_xt[3]],       [(3, 0, 0, 4), (3, 1, 0, 4)]),
            ([d_wA[(0, 0)]],  [(m, 0, 4, 10) for m in range(4)]),
            ([d_wA[(1, 0)]],  [(m, 1, 4, 10) for m in range(4)]),
            ([d_wA[(0, 1)]],  [(m, 0, 10, 16) for m in range(4)]),
            ([d_wA[(1, 1)]],  [(m, 1, 10, 16) for m in range(4)]),
        ]
        for sems, spans in waves:
            for s in sems:
                pe_wait(s)
            for m, b, k0, k1 in spans:
                for ko in range(k0, k1):
                    mm(4 * b + m, m, b, ko)

        # phase B: dense m-major k-inner; banks 0-3 (b2) free once the
        # copies of phase-A groups 0-3 are done (cp>=4), banks 4-7 (b3)
        # once groups 4-7 are copied (cp>=8) -- one hoisted wait per block
        for b in (2, 3):
            pe_wait(d_b2 if b == 2 else d_b3)
            nc.tensor.wait_ge(cp_sem, 4 if b == 2 else 8)
            for m in range(4):
                bank = (b - 2) * 4 + m
                for ko in range(KO):
                    mm(bank, m, b, ko)
    else:
        # generic fallback (never hit for the 512-tokens/expert case):
        # sequential blocks, chunk-paced, 7-bank rotation
        d_xt = [load_xt(m) for m in range(mt)]
        d_w = {}
        CH = [(0, 4), (4, 8), (8, 12), (12, 16)]
        for b in range(NBLK):
            for ci, (k0, k1) in enumerate(CH):
                d_w[(b, ci)] = load_w(b, k0, k1)
        for m in range(mt):
            pe_wait(d_xt[m])
        for b in range(NBLK):
            for m in range(mt):
                g = b * mt + m
                if g >= 7:
                    nc.tensor.wait_ge(cp_sem, g - 7 + 1)
                for ci, (k0, k1) in enumerate(CH):
                    pe_wait(d_w[(b, ci)])
                    for ko in range(k0, k1):
                        mm(g % 7, m, b, ko)

    # ---- DVE: psum -> sbuf staging (bf16); group order == stop order
    def group_bank(g):
        if mt == 4:
            return g if g < 8 else g - 8
        return g % 7

    for g in range(NG):
        nc.vector.wait_ge(pe_sem, g + 1)
        if g >= N_OSB:
            nc.vector.wait_ge(od[g - N_OSB], 16)
        nc.vector.tensor_copy(o_sb[g % N_OSB], pk[group_bank(g)]).then_inc(
            cp_sem, 1
        )

    # ---- scalar ring: output DMAs (each a contiguous 128KB block)
    for g in range(NG):
        b, m = divmod(g, mt)
        nc.scalar.wait_ge(cp_sem, g + 1)
        nc.scalar.dma_start(
            out_d[b][m * P:(m + 1) * P, :], o_sb[g % N_OSB]
        ).then_inc(od[g], 16)
    # no end-of-kernel waits on the output DMAs: the fixed walrus NEFF
    # epilogue (per-semaphore resets, ~6.5us after the final barrier) runs
    # long past the last output's completion receipt

    nc.compile()
    return nc


def _swizzle_x(x_pad: np.ndarray, t_pad: int) -> np.ndarray:
    # [t_pad, K] f32 -> [mt, P, KO*P] bf16, xt[mi,p,ko*P+j] = X[mi*P+j, ko*P+p]
    mt = t_pad // P
    v = x_pad.reshape(mt, P, KO, P).transpose(0, 3, 2, 1)
    return np.ascontiguousarray(
        v.astype(NP_COMPUTE).reshape(mt, P, KO * P))


def _swizzle_w(w_g: np.ndarray) -> np.ndarray:
    # [K, N] f32 -> [NBLK, P, KO*BW], w[b,p,ko*BW+j] = W[ko*P+p, b*BW+j]
    v = w_g.reshape(KO, P, NBLK, BW).transpose(2, 1, 0, 3)
    return np.ascontiguousarray(
        v.astype(NP_COMPUTE).reshape(NBLK, P, KO * BW))


def _run(input, weight, tokens_per_expert, trace=False, **trace_kwargs):
    inp = np.ascontiguousarray(np.asarray(input), dtype=np.float32)
    wgt = np.ascontiguousarray(np.asarray(weight), dtype=np.float32)
    counts = np.asarray(tokens_per_expert).astype(np.int64)
    num_tokens, k = inp.shape
    assert k == K and wgt.shape == (G, K, N)
    # token group boundaries (matches searchsorted(cumsum, arange, 'right')),
    # clamped to the token range for safety on degenerate counts
    ends = np.minimum(np.cumsum(counts), num_tokens)
    starts = np.minimum(ends - counts, num_tokens)
    sizes = np.maximum(ends - starts, 0)

    t_pad = max(P, int(-(-max(int(sizes.max()), 1) // P)) * P)
    nc = _build(t_pad)

    in_maps = []
    for g in range(G):
        x_pad = np.zeros((t_pad, K), dtype=np.float32)
        x_pad[: sizes[g]] = inp[starts[g]:ends[g]]
        in_maps.append({"xt": _swizzle_x(x_pad, t_pad), "w": _swizzle_w(wgt[g])})

    res = bass_utils.run_bass_kernel_spmd(
        nc, in_maps, core_ids=list(range(G)), trace=trace, **trace_kwargs
    )

    # tokens not covered by any expert group get zero output (matches the
    # reference's masked accumulation)
    out = np.zeros((num_tokens, N), dtype=np.float32)
    for g in range(G):
        blk = np.asarray(res.results[g]["out"])  # [NBLK, t_pad, BW]
        full = blk.transpose(1, 0, 2).reshape(t_pad, N)
        out[starts[g]:ends[g]] = full[: sizes[g]].astype(np.float32)
    return out, res


def kernel(input, weight, tokens_per_expert):
    out, _ = _run(input, weight, tokens_per_expert)
    return out


# revision 13
# speedup vs baseline: 1.0549x; 1.0549x over previous
"""AriaGroupedGEMM (MoE grouped GEMM) on 8 TRN2 NeuronCores.

Problem: input [4096, 2048] f32, weight [8, 2048, 2048] f32,
tokens_per_expert [8] int32 (tokens pre-sorted by expert).
out[i] = input[i] @ weight[expert_of(i)].

Strategy: expert-parallel. Core g owns expert g's weight and its token
group (boundaries computed on host from tokens_per_expert). Each core
runs a dense [T_pad, 2048] @ [2048, 2048] GEMM in bf16 (fp32 PSUM
accumulation): 256 matmuls of [128x128]@[128x512] = 54.6us of PE
streaming at the warm 2.4GHz back-to-back rate -- the compute floor.

Raw bacc (no TileContext), manual semaphores. Each HWDGE dma_start
occupies its sequencer ~0.65us and its completion semaphore lags the
last byte by ~2-3us (SDMA engine start + per-engine completion skew),
so the input stream runs on the sync ring in exact consumption order,
with the two transfers gating the first matmul split across BOTH
HWDGE rings (sync + scalar) so they complete together. Phase A
interleaves n-blocks 0/1 at k-chunk granularity across all 8 PSUM
banks (each arriving chunk unlocks 16 matmuls = 3.4us of PE work),
phase B (blocks 2/3) prefetches far ahead as two 2MB DMAs and runs
dense m-major bursts. Enough thin warm-up matmuls run over scratch to
put >3.4us of PE busy-time before the first real matmul, so the HAM
clock gate is fully lifted (2.4GHz) when data lands. Outputs stage
through 8 SBUF tiles onto the scalar ring as contiguous 128KB blocks;
the fixed walrus NEFF epilogue (~6.5us of per-semaphore resets) runs
long past the last output's completion receipt, so no end-of-kernel
completion waits are needed.
"""
import sys
import functools

for _p in ("/opt/trn_rl_repo", "/root/.axon_site/_ro/trn_rl_repo"):
    if _p not in sys.path:
        sys.path.insert(0, _p)

import numpy as np
import ml_dtypes

import concourse.mybir as mybir
from concourse import bacc
from concourse import bass_utils

P = 128
K = 2048            # in_features (contraction)
N = 2048            # out_features
G = 8               # experts == cores
KO = K // P         # 16 k-subtiles
BW = 512            # n-block width (one PSUM bank of fp32)
NBLK = N // BW      # 4 n-blocks

COMPUTE_DT = mybir.dt.bfloat16
NP_COMPUTE = ml_dtypes.bfloat16
OUT_DT = mybir.dt.bfloat16      # psum(f32) -> bf16 on the way out; host upcasts

N_WARMUP_MM = 40    # thin N=128 warm-up matmuls; >3.4us of PE busy lifts HAM
N_OSB = 8           # output staging tiles in SBUF


@functools.lru_cache(maxsize=4)
def _build(t_pad: int):
    """Build + compile the per-core GEMM graph for token-pad t_pad."""
    mt = t_pad // P  # m tiles of 128 tokens

    nc = bacc.Bacc("TRN2", target_bir_lowering=False, debug=False)

    # host-swizzled DRAM layouts (fully contiguous per DMA):
    # xt[mi, p, ko*P + j] = X[mi*P + j, ko*P + p]
    # w[b, p, ko*BW + j]  = W[ko*P + p, b*BW + j]
    # out[b, t, j]        = OUT[t, b*BW + j]
    xt_d = nc.dram_tensor(
        "xt", [mt, P, KO * P], COMPUTE_DT, kind="ExternalInput").ap()
    w_d = nc.dram_tensor(
        "w", [NBLK, P, KO * BW], COMPUTE_DT, kind="ExternalInput").ap()
    out_d = nc.dram_tensor(
        "out", [NBLK, t_pad, BW], OUT_DT, kind="ExternalOutput").ap()

    # SBUF
    xt_sb = [nc.alloc_sbuf_tensor(f"xt_sb{m}", [P, KO * P], COMPUTE_DT).ap()
             for m in range(mt)]
    w_sb = [nc.alloc_sbuf_tensor(f"w_sb{b}", [P, KO * BW], COMPUTE_DT).ap()
            for b in range(NBLK)]
    o_sb = [nc.alloc_sbuf_tensor(f"o_sb{i}", [P, BW], OUT_DT).ap()
            for i in range(N_OSB)]
    wu_lhs = nc.alloc_sbuf_tensor("wu_lhs", [P, P], COMPUTE_DT).ap()
    wu_rhs = nc.alloc_sbuf_tensor("wu_rhs", [P, P], COMPUTE_DT).ap()

    # PSUM: 8 banks; phase A owns all of them as (b, m) -> 4b+m for
    # b in {0,1}; phase B reuses bank (b-2)*4+m after its copy drains.
    # Warm-ups hit bank 7, whose first real tenant starts much later.
    pk = [nc.alloc_psum_tensor(f"pk{j}", [P, BW], mybir.dt.float32).ap()
          for j in range(8)]
    wu_ps = pk[7][:, :P]

    NG = NBLK * mt  # real matmul groups

    pe_sem = nc.alloc_semaphore("pe_sem")   # PE group-final matmul done
    cp_sem = nc.alloc_semaphore("cp_sem")   # DVE psum->sbuf copy done
    od = [nc.alloc_semaphore(f"od{g}") for g in range(NG)]  # out DMA done

    # ---- sync ring: input DMAs in exact consumption order, one sem each
    dsem = []

    def dma_in(dst_ap, src_ap, tag, ring=None):
        s = nc.alloc_semaphore(f"d{len(dsem)}_{tag}")
        (ring or nc.sync).dma_start(dst_ap, src_ap).then_inc(s, 16)
        dsem.append(s)
        return len(dsem) - 1

    def load_xt(m, k0=0, k1=KO):
        return dma_in(xt_sb[m][:, k0 * P:k1 * P],
                      xt_d[m][:, k0 * P:k1 * P], f"xt{m}_{k0}")

    def load_w(b, k0, k1, ring=None):
        return dma_in(w_sb[b][:, k0 * BW:k1 * BW],
                      w_d[b][:, k0 * BW:k1 * BW], f"w{b}_{k0}", ring)

    # ---- PE stream helpers
    waited = set()

    def pe_wait(sem_id):
        if sem_id is not None and sem_id not in waited:
            nc.tensor.wait_ge(dsem[sem_id], 16)
            waited.add(sem_id)

    def mm(bank, m, b, ko):
        ins = nc.tensor.matmul(
            pk[bank],
            xt_sb[m][:, ko * P:(ko + 1) * P],
            w_sb[b][:, ko * BW:(ko + 1) * BW],
            start=(ko == 0),
            stop=(ko == KO - 1),
        )
        if ko == KO - 1:
            ins.then_inc(pe_sem, 1)

    # warm-ups: matmuls over uninitialized scratch (result never read);
    # no deps at all, so the PE is busy from the first instant
    for _ in range(N_WARMUP_MM):
        nc.tensor.matmul(wu_ps, wu_lhs, wu_rhs, start=True, stop=True,
                         skip_group_check=True)

    if mt == 4:
        # DMA order = PE consumption order (phase A interleaves b0/b1)
        # the two gating transfers go out on both HWDGE rings in parallel;
        # the SDMA engines interleave them at packet granularity so both
        # complete together (~1us earlier than serialized on one ring)
        d_xt0a = load_xt(0, 0, 8)
        d_w0a = load_w(0, 0, 2, ring=nc.scalar)
        d_w0b = load_w(0, 2, 4)
        d_xt0b = load_xt(0, 8, 16)
        d_xt = {1: load_xt(1), 0: None}
        d_w1a = load_w(1, 0, 4)
        d_xt[2] = load_xt(2)
        d_xt[3] = load_xt(3)
        d_wA = {}  # (b, ci) -> sem, chunks k4-9, k10-15 for b0/b1
        for ci, (k0, k1) in enumerate([(4, 10), (10, 16)]):
            d_wA[(0, ci)] = load_w(0, k0, k1)
            d_wA[(1, ci)] = load_w(1, k0, k1)
        d_b2 = load_w(2, 0, 16)
        d_b3 = load_w(3, 0, 16)

        # phase A waves: (waits, [(m, b, ko-range)])
        waves = [
            ([d_xt0a, d_w0a], [(0, 0, 0, 2)]),
            ([d_w0b],         [(0, 0, 2, 4)]),
            ([d_xt[1]],       [(1, 0, 0, 4)]),
            ([d_w1a],         [(0, 1, 0, 4), (1, 1, 0, 4)]),
            ([d_xt0b],        []),
            ([d_xt[2]],       [(2, 0, 0, 4), (2, 1, 0, 4)]),
            ([d_xt[3]],       [(3, 0, 0, 4), (3, 1, 0, 4)]),
            ([d_wA[(0, 0)]],  [(m, 0, 4, 10) for m in range(4)]),
            ([d_wA[(1, 0)]],  [(m, 1, 4, 10) for m in range(4)]),
            ([d_wA[(0, 1)]],  [(m, 0, 10, 16) for m in range(4)]),
            ([d_wA[(1, 1)]],  [(m, 1, 10, 16) for m in range(4)]),
        ]
        for sems, spans in waves:
            for s in sems:
                pe_wait(s)
            for m, b, k0, k1 in spans:
                for ko in range(k0, k1):
                    mm(4 * b + m, m, b, ko)

        # phase B: dense m-major k-inner; banks 0-3 (b2) free once the
        # copies of phase-A groups 0-3 are done (cp>=4), banks 4-7 (b3)
        # once groups 4-7 are copied (cp>=8) -- one hoisted wait per block
        for b in (2, 3):
            pe_wait(d_b2 if b == 2 else d_b3)
            nc.tensor.wait_ge(cp_sem, 4 if b == 2 else 8)
            for m in range(4):
                bank = (b - 2) * 4 + m
                for ko in range(KO):
                    mm(bank, m, b, ko)
    else:
        # generic fallback (never hit for the 512-tokens/expert case):
        # sequential blocks, chunk-paced, 7-bank rotation
        d_xt = [load_xt(m) for m in range(mt)]
        d_w = {}
        CH = [(0, 4), (4, 8), (8, 12), (12, 16)]
        for b in range(NBLK):
            for ci, (k0, k1) in enumerate(CH):
                d_w[(b, ci)] = load_w(b, k0, k1)
        for m in range(mt):
            pe_wait(d_xt[m])
        for b in range(NBLK):
            for m in range(mt):
                g = b * mt + m
                if g >= 7:
                    nc.tensor.wait_ge(cp_sem, g - 7 + 1)
                for ci, (k0, k1) in enumerate(CH):
                    pe_wait(d_w[(b, ci)])
                    for ko in range(k0, k1):
                        mm(g % 7, m, b, ko)

    # ---- DVE: psum -> sbuf staging (bf16); group order == stop order
    def group_bank(g):
        if mt == 4:
            return g if g < 8 else g - 8
        return g % 7

    for g in range(NG):
        nc.vector.wait_ge(pe_sem, g + 1)
        if g >= N_OSB:
            nc.vector.wait_ge(od[g - N_OSB], 16)
        nc.vector.tensor_copy(o_sb[g % N_OSB], pk[group_bank(g)]).then_inc(
            cp_sem, 1
        )

    # ---- scalar ring: output DMAs (each a contiguous 128KB block)
    for g in range(NG):
        b, m = divmod(g, mt)
        nc.scalar.wait_ge(cp_sem, g + 1)
        nc.scalar.dma_start(
            out_d[b][m * P:(m + 1) * P, :], o_sb[g % N_OSB]
        ).then_inc(od[g], 16)
    # no end-of-kernel waits on the output DMAs: the fixed walrus NEFF
    # epilogue (per-semaphore resets, ~6.5us after the final barrier) runs
    # long past the last output's completion receipt

    nc.compile()
    return nc


def _swizzle_x(x_pad: np.ndarray, t_pad: int) -> np.ndarray:
    # [t_pad, K] f32 -> [mt, P, KO*P] bf16, xt[mi,p,ko*P+j] = X[mi*P+j, ko*P+p]
    mt = t_pad // P
    v = x_pad.reshape(mt, P, KO, P).transpose(0, 3, 2, 1)
    return np.ascontiguousarray(
        v.astype(NP_COMPUTE).reshape(mt, P, KO * P))


def _swizzle_w(w_g: np.ndarray) -> np.ndarray:
    # [K, N] f32 -> [NBLK, P, KO*BW], w[b,p,ko*BW+j] = W[ko*P+p, b*BW+j]
    v = w_g.reshape(KO, P, NBLK, BW).transpose(2, 1, 0, 3)
    return np.ascontiguousarray(
        v.astype(NP_COMPUTE).reshape(NBLK, P, KO * BW))


def _run(input, weight, tokens_per_expert, trace=False, **trace_kwargs):
    inp = np.ascontiguousarray(np.asarray(input), dtype=np.float32)
    wgt = np.ascontiguousarray(np.asarray(weight), dtype=np.float32)
    counts = np.asarray(tokens_per_expert).astype(np.int64)
    num_tokens, k = inp.shape
    assert k == K and wgt.shape == (G, K, N)
    # token group boundaries (matches searchsorted(cumsum, arange, 'right')),
    # clamped to the token range for safety on degenerate counts
    ends = np.minimum(np.cumsum(counts), num_tokens)
    starts = np.minimum(ends - counts, num_tokens)
    sizes = np.maximum(ends - starts, 0)

    t_pad = max(P, int(-(-max(int(sizes.max()), 1) // P)) * P)
    nc = _build(t_pad)

    in_maps = []
    for g in range(G):
        x_pad = np.zeros((t_pad, K), dtype=np.float32)
        x_pad[: sizes[g]] = inp[starts[g]:ends[g]]
        in_maps.append({"xt": _swizzle_x(x_pad, t_pad), "w": _swizzle_w(wgt[g])})

    res = bass_utils.run_bass_kernel_spmd(
        nc, in_maps, core_ids=list(range(G)), trace=trace, **trace_kwargs
    )

    # tokens not covered by any expert group get zero output (matches the
    # reference's masked accumulation)
    out = np.zeros((num_tokens, N), dtype=np.float32)
    for g in range(G):
        blk = np.asarray(res.results[g]["out"])  # [NBLK, t_pad, BW]
        full = blk.transpose(1, 0, 2).reshape(t_pad, N)
        out[starts[g]:ends[g]] = full[: sizes[g]].astype(np.float32)
    return out, res


def kernel(input, weight, tokens_per_expert):
    out, _ = _run(input, weight, tokens_per_expert)
    return out
